# revision 1
# baseline (speedup 1.0000x reference)
# Trainium2 Bass kernel for masked (key-padding) attention layer.
#
#   q,k,v = x@Wq, x@Wk, x@Wv ; score = q@k^T/sqrt(T) masked over keys;
#   out = softmax(score)@v @ Wo
#
# Sharding: data-parallel over batch, B=8 -> one batch element per NeuronCore,
# full weights replicated. No collectives.
#
# Per-core algorithm. Everything stays in a "transposed" layout so no on-chip
# transposes are needed, and the weight matrices are pre-folded on the host
# (A = Wq @ Wk^T and Avo = Wv @ Wo, tiny 512x512 f32 matmuls) so the device
# contracts everything through the sparse key side:
#   score[t,j] = x[t] @ A @ xv[j]^T:
#     u[x,j]  = sum_x' A[x,x'] xvT[x',j]        (48 MMs over NVP keys)
#     sT[j,t] = sum_x u[x,j] xT[x,t]            (144 MMs)
#   out = softmax @ (xv @ Avo):
#     v2[j,o] = sum_x xvT[x,j] Avo[x,o]         (36 MMs)
#     oT[o,t] = sum_j v2[j,o] eT[j,t]           (144 MMs; output projection
#                                                already folded in)
#   eT = exp(sT/sqrt(T) + kbias) (ScalarE, PSUM->SBUF bf16)
#   den[t] = sum_j eT[j,t]: DVE accumulate over j-tiles + ones-matmul partition
#     reduction -> [1,T] row; reciprocal; gpsimd partition_broadcast -> [128,T].
#   out rows: oT * recip_bc (DVE), DMA'd to DRAM as outT [D,T]; the host
#   transposes back to [T,D] when assembling the full [B,T,D] result.
import math

import numpy as np
import ml_dtypes

B = 8
T = 2048
D = 512
P = 128
KC = D // P       # 4 contraction chunks of 128
QB = 512          # free-dim chunk (one PSUM bank of f32)
NQ = T // QB      # 4 query chunks
MT = T // P       # 16 query tiles
SCALE = 1.0 / math.sqrt(float(T))
PAD_BIAS = -30000.0

_BF16 = ml_dtypes.bfloat16

_nc_cache: dict[int, object] = {}


def _build(nvp: int):
    """Build + compile the single-core SPMD program for NVP padded keys."""
    import concourse.bass as bass
    import concourse.bass_isa as bass_isa
    import concourse.mybir as mybir
    import concourse.tile as tile
    from concourse import bacc

    dt = mybir.dt
    f32, bf16 = dt.float32, dt.bfloat16
    nt = nvp // P
    kchunks = [(s, min(QB, nvp - s)) for s in range(0, nvp, QB)]

    nc = bacc.Bacc(
        "TRN2",
        target_bir_lowering=False,
        debug=False,
        enable_asserts=False,
        num_devices=B,
    )

    xT_d = nc.dram_tensor("xT", [D, T], bf16, kind="ExternalInput")
    xvT_d = nc.dram_tensor("xvT", [D, nvp], bf16, kind="ExternalInput")
    AT_d = nc.dram_tensor("AT", [D, D], bf16, kind="ExternalInput")
    Avo_d = nc.dram_tensor("Avo", [D, D], bf16, kind="ExternalInput")
    kbias_d = nc.dram_tensor("kbias", [P, nt], f32, kind="ExternalInput")
    outT_d = nc.dram_tensor("outT", [D, T], f32, kind="ExternalOutput")

    Exp = mybir.ActivationFunctionType.Exp

    with tile.TileContext(nc) as tc:
        with (
            tc.tile_pool(name="const", bufs=1) as cpool,
            tc.tile_pool(name="big", bufs=1) as bpool,
            tc.tile_pool(name="psum", bufs=8, space="PSUM") as psum,
            tc.tile_pool(name="outs", bufs=4) as opool,
        ):
            # ---- persistent SBUF tensors ----
            xT = bpool.tile([P, KC, T], bf16, tag="xT")
            xvT = bpool.tile([P, KC, nvp], bf16, tag="xvT")
            AT = cpool.tile([P, KC, D], bf16, tag="AT")
            Avo = cpool.tile([P, KC, D], bf16, tag="Avo")
            kbias = cpool.tile([P, nt], f32, tag="kbias")
            u = bpool.tile([P, KC, nvp], bf16, tag="u")
            v2 = bpool.tile([P, nt, D], bf16, tag="v2")
            eT = bpool.tile([P, nt, T], bf16, tag="eT")
            dacc = bpool.tile([P, T], f32, tag="dacc")
            rbc = bpool.tile([P, T], f32, tag="rbc")

            # ---- input loads, ordered by first use; per-dma_start issue cost
            # is substantial so batch into few DMAs, big late-needed xT last.
            xvT_src = xvT_d.ap().rearrange("(c p) t -> p c t", p=P)
            AT_src = AT_d.ap().rearrange("(c p) h -> p c h", p=P)
            nc.sync.dma_start(AT[:, 0, :], AT_src[:, 0, :])
            xc = kchunks + [None, None]
            lo, sz = xc[0]
            nc.sync.dma_start(xvT[:, :, lo : lo + sz], xvT_src[:, :, lo : lo + sz])
            for c in range(1, KC):
                nc.sync.dma_start(AT[:, c, :], AT_src[:, c, :])
            if xc[1] is not None:
                lo, sz = xc[1]
                nc.sync.dma_start(xvT[:, :, lo : lo + sz], xvT_src[:, :, lo : lo + sz])
            nc.sync.dma_start(Avo[:], Avo_d.ap().rearrange("(c p) h -> p c h", p=P))
            for blk in xc[2:]:
                if blk is None:
                    continue
                lo, sz = blk
                nc.sync.dma_start(xvT[:, :, lo : lo + sz], xvT_src[:, :, lo : lo + sz])
            nc.sync.dma_start(kbias[:], kbias_d.ap())
            nc.sync.dma_start(xT[:], xT_d.ap().rearrange("(c p) t -> p c t", p=P))
            nc.vector.memset(dacc[:], 0.0)

            # ---- stage A1: u = A @ xv^T  [x, j]; kchunk-outer, c-inner so
            # the first MMs need only AT chunk 0 + the first xvT block.
            for (s, sz) in kchunks:
                pk = [psum.tile([P, QB], f32, tag="ps", name="ps")
                      for _ in range(KC)]
                for c in range(KC):
                    for m in range(KC):
                        nc.tensor.matmul(
                            pk[m][:, :sz],
                            AT[:, c, m * P : (m + 1) * P],
                            xvT[:, c, s : s + sz],
                            start=(c == 0),
                            stop=(c == KC - 1),
                        )
                for m in range(KC):
                    nc.vector.tensor_copy(u[:, m, s : s + sz], pk[m][:, :sz])

            # ---- stage A2: v2 = xv @ Avo  [j, o] ----
            for j in range(nt):
                pv = psum.tile([P, D], f32, tag="ps", name="ps")
                for c in range(KC):
                    nc.tensor.matmul(
                        pv[:],
                        xvT[:, c, j * P : (j + 1) * P],
                        Avo[:, c, :],
                        start=(c == 0),
                        stop=(c == KC - 1),
                    )
                nc.vector.tensor_copy(v2[:, j, :], pv[:])

            # ---- stage B: scores + exp + denominator accumulation ----
            for j in range(nt):
                ps = [psum.tile([P, QB], f32, tag="ps", name="ps") for _ in range(NQ)]
                for c in range(KC):
                    for t in range(NQ):
                        nc.tensor.matmul(
                            ps[t][:],
                            u[:, c, j * P : (j + 1) * P],
                            xT[:, c, t * QB : (t + 1) * QB],
                            start=(c == 0),
                            stop=(c == KC - 1),
                        )
                for t in range(NQ):
                    sl = slice(t * QB, (t + 1) * QB)
                    nc.scalar.activation(
                        eT[:, j, sl],
                        ps[t][:],
                        Exp,
                        bias=kbias[:, j : j + 1],
                        scale=SCALE,
                    )
                    nc.vector.tensor_add(dacc[:, sl], dacc[:, sl], eT[:, j, sl])

            # ---- denominator (chunk-pipelined): gpsimd all-reduce across
            # partitions puts the broadcast column sums of dacc directly in
            # rbc (f32), then reciprocal on DVE. No PE work, no PSUM bank.
            for tt in range(NQ):
                sl = slice(tt * QB, (tt + 1) * QB)
                nc.gpsimd.partition_all_reduce(
                    rbc[:, sl], dacc[:, sl], P, bass_isa.ReduceOp.add
                )
                nc.vector.reciprocal(rbc[:, sl], rbc[:, sl])

            # ---- stage C: oT[o,t] = sum_j v2[j,o] eT[j,t], normalized and
            # written out as outT. t innermost: each [128,QB] output tile
            # completes alone and streams out (mul + DMA) immediately.
            for o in range(KC):
                for t in range(NQ):
                    po = psum.tile([P, QB], f32, tag="ps", name="ps")
                    for j in range(nt):
                        nc.tensor.matmul(
                            po[:],
                            v2[:, j, o * P : (o + 1) * P],
                            eT[:, j, t * QB : (t + 1) * QB],
                            start=(j == 0),
                            stop=(j == nt - 1),
                        )
                    ot = opool.tile([P, QB], f32, tag="ot", name="ot")
                    nc.vector.tensor_mul(
                        ot[:], po[:], rbc[:, t * QB : (t + 1) * QB]
                    )
                    nc.sync.dma_start(
                        outT_d[o * P : (o + 1) * P, t * QB : (t + 1) * QB], ot[:]
                    )

    nc.compile()
    return nc


def _get_nc(nvp: int):
    nc = _nc_cache.get(nvp)
    if nc is None:
        nc = _build(nvp)
        _nc_cache[nvp] = nc
    return nc


def _prep_inputs(x, mask, W_q, W_k, W_v, W_o):
    x = np.asarray(x, dtype=np.float32)
    mask = np.asarray(mask)
    valid = mask != 0
    nv = valid.sum(axis=1)
    nvp = int(min(T, max(P, ((int(nv.max()) + P - 1) // P) * P)))
    nt = nvp // P

    wq = np.asarray(W_q, np.float32)
    wk = np.asarray(W_k, np.float32)
    wv = np.asarray(W_v, np.float32)
    wo = np.asarray(W_o, np.float32)
    a = wq @ wk.T          # [x, x']; score = x @ A @ xv^T
    avo = wv @ wo          # [x, o];  out = attn @ xv @ Avo
    aT16 = np.ascontiguousarray(a.T).astype(_BF16)
    avo16 = np.ascontiguousarray(avo).astype(_BF16)

    in_maps = []
    for b in range(B):
        idx = np.nonzero(valid[b])[0]
        xv = np.zeros((nvp, D), np.float32)
        xv[: len(idx)] = x[b, idx]
        bias = np.full((nvp,), PAD_BIAS, np.float32)
        bias[: len(idx)] = 0.0
        in_maps.append(
            {
                "xT": np.ascontiguousarray(x[b].T).astype(_BF16),
                "xvT": np.ascontiguousarray(xv.T).astype(_BF16),
                "AT": aT16,
                "Avo": avo16,
                "kbias": np.ascontiguousarray(bias.reshape(nt, P).T),
            }
        )
    return nvp, in_maps


def _run(x, mask, W_q, W_k, W_v, W_o, trace=False):
    from concourse.bass_utils import run_bass_kernel_spmd

    nvp, in_maps = _prep_inputs(x, mask, W_q, W_k, W_v, W_o)
    nc = _get_nc(nvp)
    res = run_bass_kernel_spmd(nc, in_maps, core_ids=list(range(B)), trace=trace)
    out = np.stack([res.results[b]["outT"].T for b in range(B)]).astype(np.float32)
    return out, res


def kernel(x, mask, W_q, W_k, W_v, W_o):
    out, _ = _run(x, mask, W_q, W_k, W_v, W_o)
    return out


# ---------------------------------------------------------------------------
# Timing helper (not used by the grading harness): replicates
# bass2jax.run_bass_via_pjrt but caches the jitted executable so repeated
# calls measure device execution without re-tracing/compiling.
def _make_runner(nc):
    import jax
    import numpy as np
    from jax.experimental.shard_map import shard_map
    from jax.sharding import Mesh, PartitionSpec
    import concourse.mybir as mybir
    from concourse import bass2jax

    bass2jax.install_neuronx_cc_hook()
    n_cores = B
    partition_name = nc.partition_id_tensor.name if nc.partition_id_tensor else None
    in_names, out_names, out_avals, zero_outs = [], [], [], []
    for alloc in nc.m.functions[0].allocations:
        if not isinstance(alloc, mybir.MemoryLocationSet):
            continue
        name = alloc.memorylocations[0].name
        if alloc.kind == "ExternalInput":
            if name != partition_name:
                in_names.append(name)
        elif alloc.kind == "ExternalOutput":
            out_names.append(name)
            shape = tuple(alloc.tensor_shape)
            dtype = mybir.dt.np(alloc.dtype)
            out_avals.append(jax.core.ShapedArray(shape, dtype))
            zero_outs.append(np.zeros(shape, dtype))
    n_params = len(in_names)
    n_outs = len(out_avals)
    all_names = in_names + out_names
    if partition_name is not None:
        all_names = all_names + [partition_name]
    donate = tuple(range(n_params, n_params + n_outs))

    def _body(*args):
        operands = list(args)
        if partition_name is not None:
            operands.append(bass2jax.partition_id_tensor())
        outs = bass2jax._bass_exec_p.bind(
            *operands,
            out_avals=tuple(out_avals),
            in_names=tuple(all_names),
            out_names=tuple(out_names),
            lowering_input_output_aliases=(),
            sim_require_finite=True,
            sim_require_nnan=True,
            nc=nc,
        )
        return tuple(outs)

    devices = jax.devices()[:n_cores]
    mesh = Mesh(np.asarray(devices), ("core",))
    in_specs = (PartitionSpec("core"),) * (n_params + n_outs)
    out_specs = (PartitionSpec("core"),) * len(out_names)
    sharded = jax.jit(
        shard_map(_body, mesh=mesh, in_specs=in_specs, out_specs=out_specs,
                  check_rep=False),
        donate_argnums=donate,
        keep_unused=True,
    )

    def run(in_maps, reps=1):
        import time
        concat_in = [
            np.concatenate([np.asarray(in_maps[c][name]) for c in range(n_cores)], axis=0)
            for name in in_names
        ]
        times = []
        outs = None
        for _ in range(reps):
            concat_zeros = [
                np.zeros((n_cores * z.shape[0], *z.shape[1:]), z.dtype)
                for z in zero_outs
            ]
            t0 = time.perf_counter()
            outs = sharded(*concat_in, *concat_zeros)
            for o in outs:
                o.block_until_ready()
            times.append(time.perf_counter() - t0)
        results = [
            {name: np.asarray(outs[i]).reshape(n_cores, *out_avals[i].shape)[c]
             for i, name in enumerate(out_names)}
            for c in range(n_cores)
        ]
        return results, times

    return run

